# revision 1
# baseline (speedup 1.0000x reference)
"""Trainium2 Bass kernel for nn_ContextualCritic (4-layer strided conv + segment mean).

Self-contained: kernel(**inputs) -> np.ndarray [2B, 8192].

Design (per core, data-parallel over 8 cores, 512 images each):
 - L1 (3->64, 5x5 s2): host im2col to K=75, zero-padded to K=128; weights
   duplicated along M so the output lands twice in PSUM partitions (0-63 and
   64-127) -- this materializes the partition-duplicate the L2 row-group
   pairing needs for free.
 - L2 (64->128): 25 taps as interleaved K=64 matmul pairs on PE row groups
   (0,0)/(64,0) into two PSUM banks (full xbus-level overlap), plus the odd
   tap as one K=128 matmul with zeroed high weight rows; DVE adds banks,
   ACT applies bias+LeakyReLU into the padded L3 input layout.
 - L3 (128->256), L4 (256->512): direct K=128 accumulation matmuls over taps
   (x2 ci planes for L4), fp32r everywhere (1 cyc/row at N=512).
 - Segment mean on host from [N,8192] features (sorted segment ids).
"""
import os
import numpy as np

BLK = 8        # images per Phase-A block
GRP = 32       # images per L4 group (N = GRP*16 = 512)
NCORES = 8

_CACHE = {}


def _build_program(nimg, debug=False):
    from concourse import bacc, mybir
    import concourse.tile as tile

    F32R = mybir.dt.float32r
    F32 = mybir.dt.float32
    LRELU = mybir.ActivationFunctionType.Prelu

    nblk = nimg // BLK
    ngrp = nimg // GRP

    nc = bacc.Bacc(None, target_bir_lowering=False)

    icd = nc.dram_tensor("ic", [75, nimg * 1024], F32R, kind="ExternalInput")
    w1d = nc.dram_tensor("w1", [128, 128], F32R, kind="ExternalInput")
    zd = nc.dram_tensor("zz", [128, 10368], F32R, kind="ExternalInput")
    w2d = nc.dram_tensor("w2", [128, 25 * 128], F32R, kind="ExternalInput")
    w3d = nc.dram_tensor("w3", [128, 2 * 25 * 128], F32R, kind="ExternalInput")
    w4d = nc.dram_tensor("w4", [2 * 25 * 128, 512], F32R, kind="ExternalInput")
    b1d = nc.dram_tensor("b1", [128, 1], F32, kind="ExternalInput")
    b2d = nc.dram_tensor("b2", [128, 1], F32, kind="ExternalInput")
    b3d = nc.dram_tensor("b3", [128, 2], F32, kind="ExternalInput")
    b4d = nc.dram_tensor("b4", [128, 4], F32, kind="ExternalInput")
    fd = nc.dram_tensor("f", [128, 4, nimg, 16], F32, kind="ExternalOutput")
    if debug:
        d1d = nc.dram_tensor("d1", [128, BLK, 36, 36], F32R, kind="ExternalOutput")
        d2d = nc.dram_tensor("d2", [128, BLK, 16, 16], F32R, kind="ExternalOutput")
        d3d = nc.dram_tensor("d3", [128, GRP, 12, 12], F32R, kind="ExternalOutput")

    with tile.TileContext(nc) as tc:
        with tc.tile_pool(name="const", bufs=1) as cst, \
             tc.tile_pool(name="dram", bufs=1, space="DRAM") as drp:
            l2od = drp.tile([128, nimg, 16, 16], F32R)

            w1t = cst.tile([128, 128], F32R)
            nc.sync.dma_start(w1t[:], w1d[:, :])
            w2t = cst.tile([128, 25 * 128], F32R)
            nc.sync.dma_start(w2t[:], w2d[:, :])
            w3t = cst.tile([128, 2 * 25 * 128], F32R)
            nc.sync.dma_start(w3t[:], w3d[:, :])
            b1t = cst.tile([128, 1], F32)
            nc.sync.dma_start(b1t[:], b1d[:, :])
            b2t = cst.tile([128, 1], F32)
            nc.sync.dma_start(b2t[:], b2d[:, :])
            b3t = cst.tile([128, 2], F32)
            nc.sync.dma_start(b3t[:], b3d[:, :])
            b4t = cst.tile([128, 4], F32)
            nc.sync.dma_start(b4t[:], b4d[:, :])
            a2t = cst.tile([128, 1], F32)
            nc.vector.memset(a2t[:], 0.2)

            # ---------------- Phase A: L1 + L2 ----------------
            with tc.tile_pool(name="pa", bufs=1) as pa, \
                 tc.tile_pool(name="paps", bufs=2, space="PSUM") as paps, \
                 tc.tile_pool(name="past", bufs=3) as past:
                icT = [pa.tile([128, BLK * 1024], F32R, name=f"ic{i}")
                       for i in range(2)]
                l2iT = [pa.tile([128, BLK, 36, 36], F32R, name=f"l2i{i}")
                        for i in range(2)]
                for i in range(2):
                    nc.sync.dma_start(icT[i][75:128, :], zd[0:53, 0:BLK * 1024])
                    for im in range(BLK):
                        nc.sync.dma_start(l2iT[i][:, im, :, :], zd[:, 0:1296])

                for blk in range(nblk):
                    ic = icT[blk % 2]
                    l2i = l2iT[blk % 2]
                    c0 = blk * BLK * 1024
                    nc.sync.dma_start(ic[0:64, :], icd[0:64, c0:c0 + BLK * 1024])
                    nc.sync.dma_start(ic[64:75, :], icd[64:75, c0:c0 + BLK * 1024])
                    # L1: 16 psum blocks of 512 out pixels (half image each)
                    for psb in range(16):
                        img, h = psb // 2, psb % 2
                        ps = paps.tile([128, 16, 32], F32, tag="l1ps")
                        nc.tensor.matmul(ps[:, :, :], w1t[:, :],
                                         ic[:, psb * 512:(psb + 1) * 512],
                                         start=True, stop=True)
                        nc.scalar.activation(
                            l2i[:, img, 2 + 16 * h:18 + 16 * h, 2:34],
                            ps[:, :, :], LRELU, bias=b1t[:, :], alpha=a2t[:, :])
                    # L2: 4 psum blocks of 512 out pixels (2 images each)
                    for psb in range(4):
                        j0 = 2 * psb
                        psA = paps.tile([128, 512], F32, tag="l2psA")
                        psB = paps.tile([128, 512], F32, tag="l2psB")
                        for i in range(12):
                            tA, tB = 2 * i, 2 * i + 1
                            ka, wa = tA // 5, tA % 5
                            kb, wb = tB // 5, tB % 5
                            nc.tensor.matmul(
                                psA[:, :], w2t[0:64, tA * 128:(tA + 1) * 128],
                                l2i[0:64, j0:j0 + 2, ka:ka + 32:2,
                                    wa:wa + 32:2],
                                start=(i == 0), stop=False)
                            nc.tensor.matmul(
                                psB[:, :], w2t[64:128, tB * 128:(tB + 1) * 128],
                                l2i[64:128, j0:j0 + 2, kb:kb + 32:2,
                                    wb:wb + 32:2],
                                start=(i == 0), stop=(i == 11),
                                tile_position=(64, 0))
                        # tap 24 as K=128 (high weight rows are zero on host)
                        nc.tensor.matmul(
                            psA[:, :], w2t[:, 24 * 128:25 * 128],
                            l2i[:, j0:j0 + 2, 4:36:2, 4:36:2],
                            start=False, stop=True)
                        tb = past.tile([128, 512], F32, tag="l2tb")
                        nc.vector.tensor_copy(tb[:], psB[:, :])
                        st = past.tile([128, 512], F32, tag="l2st")
                        nc.vector.tensor_tensor(st[:], psA[:, :], tb[:],
                                                op=mybir.AluOpType.add)
                        ob = past.tile([128, 2, 16, 16], F32R, tag="l2ob")
                        nc.scalar.activation(ob[:], st[:].rearrange(
                            "p (i r c) -> p i r c", i=2, r=16),
                            LRELU, bias=b2t[:, :], alpha=a2t[:, :])
                        nc.sync.dma_start(
                            l2od[:, blk * BLK + j0:blk * BLK + j0 + 2, :, :],
                            ob[:])
                    if debug and blk == 0:
                        for im in range(BLK):
                            nc.sync.dma_start(d1d[:, im, :, :],
                                              l2iT[0][:, im, :, :])
                        for im in range(BLK):
                            nc.sync.dma_start(d2d[:, im, :, :],
                                              l2od[:, im, :, :])

            # ---------------- Phase B: L3 + L4 ----------------
            with tc.tile_pool(name="pb", bufs=1) as pb, \
                 tc.tile_pool(name="pbps", bufs=1, space="PSUM") as pbps, \
                 tc.tile_pool(name="pbst", bufs=3) as pbst, \
                 tc.tile_pool(name="w4p", bufs=4) as w4p:
                l3iT = [pb.tile([128, BLK, 20, 20], F32R, name=f"l3i{i}")
                        for i in range(2)]
                l4iT = [pb.tile([128, GRP, 12, 12], F32R, name=f"l4i{i}")
                        for i in range(2)]
                for i in range(2):
                    for im in range(BLK):
                        nc.sync.dma_start(l3iT[i][:, im, :, :], zd[:, 0:400])
                    for im in range(GRP):
                        nc.sync.dma_start(l4iT[i][:, im, :, :], zd[:, 0:144])
                l4ps = [pbps.tile([128, GRP, 4, 4], F32, name=f"l4ps{q}")
                        for q in range(4)]

                for grp in range(ngrp):
                    for sb4 in range(4):
                        gb = grp * 4 + sb4
                        l3i = l3iT[gb % 2]
                        i0 = grp * GRP + sb4 * BLK
                        for im in range(BLK):
                            nc.sync.dma_start(l3i[:, im, 2:18, 2:18],
                                              l2od[:, i0 + im, :, :])
                        for cp in range(2):
                            ps3 = pbps.tile([128, BLK, 8, 8], F32,
                                            tag=f"l3ps{cp}")
                            for tap in range(25):
                                kh, kw = tap // 5, tap % 5
                                nc.tensor.matmul(
                                    ps3[:, :, :, :],
                                    w3t[:, (cp * 25 + tap) * 128:
                                        (cp * 25 + tap + 1) * 128],
                                    l3i[:, :, kh:kh + 16:2,
                                        kw:kw + 16:2],
                                    start=(tap == 0), stop=(tap == 24))
                            nc.scalar.activation(
                                l4iT[cp][:, sb4 * BLK:(sb4 + 1) * BLK,
                                         2:10, 2:10],
                                ps3[:, :, :, :], LRELU,
                                bias=b3t[:, cp:cp + 1], alpha=a2t[:, :])
                    if debug and grp == 0:
                        for im in range(GRP):
                            nc.sync.dma_start(d3d[:, im, :, :],
                                              l4iT[0][:, im, :, :])
                    # L4 over the 32-image group
                    for i4 in range(50):
                        cip, tap = i4 // 25, i4 % 25
                        kh, kw = tap // 5, tap % 5
                        wt = w4p.tile([128, 512], F32R, tag="w4t")
                        r0 = (cip * 25 + tap) * 128
                        nc.sync.dma_start(wt[:], w4d[r0:r0 + 128, :])
                        for q in range(4):
                            nc.tensor.matmul(
                                l4ps[q][:, :, :, :],
                                wt[:, q * 128:(q + 1) * 128],
                                l4iT[cip][:, :, kh:kh + 8:2,
                                          kw:kw + 8:2],
                                start=(i4 == 0), stop=(i4 == 49))
                    for q in range(4):
                        fo = pbst.tile([128, GRP, 16], F32, tag="fo")
                        nc.scalar.activation(fo[:], l4ps[q][:, :, :, :].rearrange(
                            "p i a b -> p i (a b)"),
                            LRELU, bias=b4t[:, q:q + 1], alpha=a2t[:, :])
                        nc.sync.dma_start(
                            fd[:, q, grp * GRP:(grp + 1) * GRP, :], fo[:])
    nc.compile()
    return nc


def _prep_inputs(x, W1, b1, W2, b2, W3, b3, W4, b4, nimg):
    """Host preprocessing -> per-core in_maps (shared weight arrays)."""
    f32 = np.float32
    n = x.shape[0]
    ncores = n // nimg
    xpad = np.pad(np.asarray(x, dtype=f32), ((0, 0), (0, 0), (2, 2), (2, 2)))
    s = xpad.strides
    v = np.lib.stride_tricks.as_strided(
        xpad, shape=(n, 3, 5, 5, 32, 32),
        strides=(s[0], s[1], s[2], s[3], 2 * s[2], 2 * s[3]))
    # [75, n, 1024]
    ic_all = np.ascontiguousarray(
        v.transpose(1, 2, 3, 0, 4, 5).reshape(75, n, 1024))

    w1l = np.ascontiguousarray(
        np.asarray(W1, f32).transpose(1, 2, 3, 0).reshape(75, 64))
    w1h = np.zeros((128, 128), f32)
    w1h[0:75, 0:64] = w1l
    w1h[0:75, 64:128] = w1l
    zz = np.zeros((128, 10368), f32)
    b1h = np.concatenate([b1, b1]).astype(f32).reshape(128, 1)

    w2h = np.zeros((128, 25 * 128), f32)
    for t in range(25):
        kh, kw = t // 5, t % 5
        lhs = np.asarray(W2, f32)[:, :, kh, kw].T                # [64,128]
        w2h[0:64, t * 128:(t + 1) * 128] = lhs
        if t < 24:
            w2h[64:128, t * 128:(t + 1) * 128] = lhs
    b2h = np.asarray(b2, f32).reshape(128, 1)

    w3h = np.zeros((128, 2 * 25 * 128), f32)
    for cp in range(2):
        for t in range(25):
            kh, kw = t // 5, t % 5
            w3h[:, (cp * 25 + t) * 128:(cp * 25 + t + 1) * 128] = \
                np.asarray(W3, f32)[cp * 128:(cp + 1) * 128, :, kh, kw].T
    b3h = np.ascontiguousarray(
        np.asarray(b3, f32).reshape(2, 128).T)                   # [128,2]

    w4h = np.zeros((2 * 25 * 128, 512), f32)
    for cip in range(2):
        for t in range(25):
            kh, kw = t // 5, t % 5
            w4h[(cip * 25 + t) * 128:(cip * 25 + t + 1) * 128, :] = \
                np.asarray(W4, f32)[:, cip * 128:(cip + 1) * 128, kh, kw].T
    b4h = np.ascontiguousarray(
        np.asarray(b4, f32).reshape(4, 128).T)                   # [128,4]

    in_maps = []
    for c in range(ncores):
        ic = np.ascontiguousarray(
            ic_all[:, c * nimg:(c + 1) * nimg, :].reshape(75, nimg * 1024))
        in_maps.append({"ic": ic, "w1": w1h, "w2": w2h, "w3": w3h,
                        "w4": w4h, "b1": b1h, "b2": b2h, "b3": b3h,
                        "b4": b4h, "zz": zz})
    return in_maps


def _run(inputs, trace=False, nimg=512, ncores=NCORES):
    from concourse.bass_utils import run_bass_kernel_spmd

    key = (nimg, ncores)
    if key not in _CACHE:
        _CACHE[key] = _build_program(nimg)
    nc = _CACHE[key]

    in_maps = _prep_inputs(
        inputs["x"], inputs["W1"], inputs["b1"], inputs["W2"], inputs["b2"],
        inputs["W3"], inputs["b3"], inputs["W4"], inputs["b4"], nimg)

    res = run_bass_kernel_spmd(nc, in_maps, core_ids=list(range(ncores)),
                               trace=trace)
    feats = np.concatenate(
        [r["f"].transpose(2, 1, 0, 3).reshape(nimg, 8192)
         for r in res.results], axis=0)                          # [N, 8192]
    return feats, res


def kernel(**inputs):
    x = np.asarray(inputs["x"])
    n = x.shape[0]
    nimg = n // NCORES
    feats, _ = _run(inputs, trace=False, nimg=nimg)

    if int(np.asarray(inputs.get("is_local", 1))) == 0:
        return feats.astype(np.float32)

    batch_size = int(np.asarray(inputs["batch_size"]))
    seg = np.asarray(inputs["f_obj_to_img"]).astype(np.int64)
    nh = n // 2
    fake, real = feats[:nh], feats[nh:]
    counts = np.bincount(seg, minlength=batch_size).astype(np.float32)
    denom = np.maximum(counts, 1.0)[:, None]
    fsum = np.zeros((batch_size, 8192), np.float32)
    rsum = np.zeros((batch_size, 8192), np.float32)
    np.add.at(fsum, seg, fake)
    np.add.at(rsum, seg, real)
    favg = np.where((counts > 0)[:, None], fsum / denom, 0.0)
    ravg = np.where((counts > 0)[:, None], rsum / denom, 0.0)
    return np.concatenate([favg, ravg], axis=0).astype(np.float32)



# revision 2
# speedup vs baseline: 1.4954x; 1.4954x over previous
"""Trainium2 Bass kernel for nn_ContextualCritic (4-layer strided conv + segment mean).

Self-contained: kernel(**inputs) -> np.ndarray [2B, 8192].

Design (per core, data-parallel over 8 cores, 512 images each), bf16 matmuls:
 - L1 (3->64, 5x5 s2): host im2col to K=75, zero-padded to K=128; weights
   duplicated along M so the output lands twice in PSUM partitions (0-63 and
   64-127) -- this materializes the partition-duplicate the L2 row-group
   pairing needs for free.
 - L2 (64->128): 25 taps as interleaved K=64 matmul pairs on PE row groups
   (0,0)/(64,0) into two PSUM banks (full xbus-level overlap), plus the odd
   tap as one K=128 matmul with zeroed high weight rows; DVE adds banks,
   ACT applies bias+LeakyReLU into the padded L3 input layout.
 - L3 (128->256), L4 (256->512): direct K=128 accumulation matmuls over taps
   (x2 ci planes for L4); w4 stays resident in SBUF (loaded once).
 - All matmul operands bf16 (fp32 PSUM accumulate; FWL active), biases fp32,
   final features fp32.
 - Segment mean on host from [N,8192] features (sorted segment ids).
"""
import os
import numpy as np

BLK = 8        # images per Phase-A block
GRP = 32       # images per L4 group (N = GRP*16 = 512)
NCORES = 8

_CACHE = {}


def _build_program(nimg, debug=False):
    from concourse import bacc, mybir
    import concourse.tile as tile

    BF16 = mybir.dt.bfloat16
    F32 = mybir.dt.float32
    LRELU = mybir.ActivationFunctionType.Prelu

    nblk = nimg // BLK
    ngrp = nimg // GRP

    nc = bacc.Bacc(None, target_bir_lowering=False)

    icd = nc.dram_tensor("ic", [75, nimg * 1024], BF16, kind="ExternalInput")
    w1d = nc.dram_tensor("w1", [128, 128], BF16, kind="ExternalInput")
    zd = nc.dram_tensor("zz", [128, 10368], BF16, kind="ExternalInput")
    w2d = nc.dram_tensor("w2", [128, 25 * 128], BF16, kind="ExternalInput")
    w3d = nc.dram_tensor("w3", [128, 2 * 25 * 128], BF16, kind="ExternalInput")
    w4d = nc.dram_tensor("w4", [128, 50 * 512], BF16, kind="ExternalInput")
    b1d = nc.dram_tensor("b1", [128, 1], F32, kind="ExternalInput")
    b2d = nc.dram_tensor("b2", [128, 1], F32, kind="ExternalInput")
    b3d = nc.dram_tensor("b3", [128, 2], F32, kind="ExternalInput")
    b4d = nc.dram_tensor("b4", [128, 4], F32, kind="ExternalInput")
    fd = nc.dram_tensor("f", [128, 4, nimg, 16], F32, kind="ExternalOutput")
    if debug:
        d1d = nc.dram_tensor("d1", [128, BLK, 36, 36], BF16, kind="ExternalOutput")
        d2d = nc.dram_tensor("d2", [128, BLK, 16, 16], BF16, kind="ExternalOutput")
        d3d = nc.dram_tensor("d3", [128, GRP, 12, 12], BF16, kind="ExternalOutput")

    with tile.TileContext(nc) as tc:
        with tc.tile_pool(name="const", bufs=1) as cst, \
             tc.tile_pool(name="dram", bufs=1, space="DRAM") as drp:
            l2od = drp.tile([128, nimg, 16, 16], BF16)

            w1t = cst.tile([128, 128], BF16)
            nc.sync.dma_start(w1t[:], w1d[:, :])
            w2t = cst.tile([128, 25 * 128], BF16)
            nc.sync.dma_start(w2t[:], w2d[:, :])
            w3t = cst.tile([128, 2 * 25 * 128], BF16)
            nc.sync.dma_start(w3t[:], w3d[:, :])
            w4t = cst.tile([128, 50 * 512], BF16)
            nc.sync.dma_start(w4t[:], w4d[:, :])
            b1t = cst.tile([128, 1], F32)
            nc.sync.dma_start(b1t[:], b1d[:, :])
            b2t = cst.tile([128, 1], F32)
            nc.sync.dma_start(b2t[:], b2d[:, :])
            b3t = cst.tile([128, 2], F32)
            nc.sync.dma_start(b3t[:], b3d[:, :])
            b4t = cst.tile([128, 4], F32)
            nc.sync.dma_start(b4t[:], b4d[:, :])
            a2t = cst.tile([128, 1], F32)
            nc.vector.memset(a2t[:], 0.2)

            # ---------------- Phase A: L1 + L2 ----------------
            with tc.tile_pool(name="pa", bufs=1) as pa, \
                 tc.tile_pool(name="paps", bufs=2, space="PSUM") as paps, \
                 tc.tile_pool(name="past", bufs=3) as past:
                icT = [pa.tile([128, BLK * 1024], BF16, name=f"ic{i}")
                       for i in range(2)]
                l2iT = [pa.tile([128, BLK, 36, 36], BF16, name=f"l2i{i}")
                        for i in range(2)]
                for i in range(2):
                    nc.sync.dma_start(icT[i][75:128, :], zd[0:53, 0:BLK * 1024])
                    for im in range(BLK):
                        nc.sync.dma_start(l2iT[i][:, im, :, :], zd[:, 0:1296])

                for blk in range(nblk):
                    ic = icT[blk % 2]
                    l2i = l2iT[blk % 2]
                    c0 = blk * BLK * 1024
                    nc.sync.dma_start(ic[0:64, :], icd[0:64, c0:c0 + BLK * 1024])
                    nc.sync.dma_start(ic[64:75, :], icd[64:75, c0:c0 + BLK * 1024])
                    # L1: 16 psum blocks of 512 out pixels (half image each)
                    for psb in range(16):
                        img, h = psb // 2, psb % 2
                        ps = paps.tile([128, 16, 32], F32, tag="l1ps")
                        nc.tensor.matmul(ps[:, :, :], w1t[:, :],
                                         ic[:, psb * 512:(psb + 1) * 512],
                                         start=True, stop=True)
                        nc.scalar.activation(
                            l2i[:, img, 2 + 16 * h:18 + 16 * h, 2:34],
                            ps[:, :, :], LRELU, bias=b1t[:, :], alpha=a2t[:, :])
                    # L2: 4 psum blocks of 512 out pixels (2 images each)
                    for psb in range(4):
                        j0 = 2 * psb
                        psA = paps.tile([128, 512], F32, tag="l2psA")
                        psB = paps.tile([128, 512], F32, tag="l2psB")
                        for i in range(12):
                            tA, tB = 2 * i, 2 * i + 1
                            ka, wa = tA // 5, tA % 5
                            kb, wb = tB // 5, tB % 5
                            nc.tensor.matmul(
                                psA[:, :], w2t[0:64, tA * 128:(tA + 1) * 128],
                                l2i[0:64, j0:j0 + 2, ka:ka + 32:2,
                                    wa:wa + 32:2],
                                start=(i == 0), stop=False)
                            nc.tensor.matmul(
                                psB[:, :], w2t[64:128, tB * 128:(tB + 1) * 128],
                                l2i[64:128, j0:j0 + 2, kb:kb + 32:2,
                                    wb:wb + 32:2],
                                start=(i == 0), stop=(i == 11),
                                tile_position=(64, 0))
                        # tap 24 as K=128 (high weight rows are zero on host)
                        nc.tensor.matmul(
                            psA[:, :], w2t[:, 24 * 128:25 * 128],
                            l2i[:, j0:j0 + 2, 4:36:2, 4:36:2],
                            start=False, stop=True)
                        tb = past.tile([128, 512], F32, tag="l2tb")
                        nc.vector.tensor_copy(tb[:], psB[:, :])
                        st = past.tile([128, 512], F32, tag="l2st")
                        nc.vector.tensor_tensor(st[:], psA[:, :], tb[:],
                                                op=mybir.AluOpType.add)
                        ob = past.tile([128, 2, 16, 16], BF16, tag="l2ob")
                        nc.scalar.activation(ob[:], st[:].rearrange(
                            "p (i r c) -> p i r c", i=2, r=16),
                            LRELU, bias=b2t[:, :], alpha=a2t[:, :])
                        nc.sync.dma_start(
                            l2od[:, blk * BLK + j0:blk * BLK + j0 + 2, :, :],
                            ob[:])
                    if debug and blk == 0:
                        for im in range(BLK):
                            nc.sync.dma_start(d1d[:, im, :, :],
                                              l2iT[0][:, im, :, :])
                        for im in range(BLK):
                            nc.sync.dma_start(d2d[:, im, :, :],
                                              l2od[:, im, :, :])

            # ---------------- Phase B: L3 + L4 ----------------
            with tc.tile_pool(name="pb", bufs=1) as pb, \
                 tc.tile_pool(name="pbps", bufs=1, space="PSUM") as pbps, \
                 tc.tile_pool(name="pbst", bufs=3) as pbst:
                l3iT = [pb.tile([128, BLK, 20, 20], BF16, name=f"l3i{i}")
                        for i in range(2)]
                l4iT = [pb.tile([128, GRP, 12, 12], BF16, name=f"l4i{i}")
                        for i in range(2)]
                for i in range(2):
                    for im in range(BLK):
                        nc.sync.dma_start(l3iT[i][:, im, :, :], zd[:, 0:400])
                    for im in range(GRP):
                        nc.sync.dma_start(l4iT[i][:, im, :, :], zd[:, 0:144])
                l4ps = [pbps.tile([128, GRP, 4, 4], F32, name=f"l4ps{q}")
                        for q in range(4)]

                for grp in range(ngrp):
                    for sb4 in range(4):
                        gb = grp * 4 + sb4
                        l3i = l3iT[gb % 2]
                        i0 = grp * GRP + sb4 * BLK
                        for im in range(BLK):
                            nc.sync.dma_start(l3i[:, im, 2:18, 2:18],
                                              l2od[:, i0 + im, :, :])
                        for cp in range(2):
                            ps3 = pbps.tile([128, BLK, 8, 8], F32,
                                            tag=f"l3ps{cp}")
                            for tap in range(25):
                                kh, kw = tap // 5, tap % 5
                                nc.tensor.matmul(
                                    ps3[:, :, :, :],
                                    w3t[:, (cp * 25 + tap) * 128:
                                        (cp * 25 + tap + 1) * 128],
                                    l3i[:, :, kh:kh + 16:2,
                                        kw:kw + 16:2],
                                    start=(tap == 0), stop=(tap == 24))
                            nc.scalar.activation(
                                l4iT[cp][:, sb4 * BLK:(sb4 + 1) * BLK,
                                         2:10, 2:10],
                                ps3[:, :, :, :], LRELU,
                                bias=b3t[:, cp:cp + 1], alpha=a2t[:, :])
                    if debug and grp == 0:
                        for im in range(GRP):
                            nc.sync.dma_start(d3d[:, im, :, :],
                                              l4iT[0][:, im, :, :])
                    # L4 over the 32-image group (w4 resident in SBUF)
                    for i4 in range(50):
                        cip, tap = i4 // 25, i4 % 25
                        kh, kw = tap // 5, tap % 5
                        for q in range(4):
                            nc.tensor.matmul(
                                l4ps[q][:, :, :, :],
                                w4t[:, i4 * 512 + q * 128:
                                    i4 * 512 + (q + 1) * 128],
                                l4iT[cip][:, :, kh:kh + 8:2,
                                          kw:kw + 8:2],
                                start=(i4 == 0), stop=(i4 == 49))
                    for q in range(4):
                        fo = pbst.tile([128, GRP, 16], F32, tag="fo")
                        nc.scalar.activation(fo[:], l4ps[q][:, :, :, :].rearrange(
                            "p i a b -> p i (a b)"),
                            LRELU, bias=b4t[:, q:q + 1], alpha=a2t[:, :])
                        nc.sync.dma_start(
                            fd[:, q, grp * GRP:(grp + 1) * GRP, :], fo[:])
    nc.compile()
    return nc


def _prep_inputs(x, W1, b1, W2, b2, W3, b3, W4, b4, nimg):
    """Host preprocessing -> per-core in_maps (shared weight arrays)."""
    import ml_dtypes
    f32 = np.float32
    bf16 = ml_dtypes.bfloat16
    n = x.shape[0]
    ncores = n // nimg
    xpad = np.pad(np.asarray(x, dtype=f32), ((0, 0), (0, 0), (2, 2), (2, 2)))
    s = xpad.strides
    v = np.lib.stride_tricks.as_strided(
        xpad, shape=(n, 3, 5, 5, 32, 32),
        strides=(s[0], s[1], s[2], s[3], 2 * s[2], 2 * s[3]))
    # [75, n, 1024]
    ic_all = np.ascontiguousarray(
        v.transpose(1, 2, 3, 0, 4, 5).reshape(75, n, 1024)).astype(bf16)

    w1l = np.ascontiguousarray(
        np.asarray(W1, f32).transpose(1, 2, 3, 0).reshape(75, 64))
    w1h = np.zeros((128, 128), f32)
    w1h[0:75, 0:64] = w1l
    w1h[0:75, 64:128] = w1l
    zz = np.zeros((128, 10368), bf16)
    b1h = np.concatenate([b1, b1]).astype(f32).reshape(128, 1)

    w2h = np.zeros((128, 25 * 128), f32)
    for t in range(25):
        kh, kw = t // 5, t % 5
        lhs = np.asarray(W2, f32)[:, :, kh, kw].T                # [64,128]
        w2h[0:64, t * 128:(t + 1) * 128] = lhs
        if t < 24:
            w2h[64:128, t * 128:(t + 1) * 128] = lhs
    b2h = np.asarray(b2, f32).reshape(128, 1)

    w3h = np.zeros((128, 2 * 25 * 128), f32)
    for cp in range(2):
        for t in range(25):
            kh, kw = t // 5, t % 5
            w3h[:, (cp * 25 + t) * 128:(cp * 25 + t + 1) * 128] = \
                np.asarray(W3, f32)[cp * 128:(cp + 1) * 128, :, kh, kw].T
    b3h = np.ascontiguousarray(
        np.asarray(b3, f32).reshape(2, 128).T)                   # [128,2]

    w4h = np.zeros((128, 50 * 512), f32)
    for cip in range(2):
        for t in range(25):
            kh, kw = t // 5, t % 5
            i4 = cip * 25 + t
            w4h[:, i4 * 512:(i4 + 1) * 512] = \
                np.asarray(W4, f32)[:, cip * 128:(cip + 1) * 128, kh, kw].T
    b4h = np.ascontiguousarray(
        np.asarray(b4, f32).reshape(4, 128).T)                   # [128,4]

    w1h = w1h.astype(bf16)
    w2h = w2h.astype(bf16)
    w3h = w3h.astype(bf16)
    w4h = w4h.astype(bf16)

    in_maps = []
    for c in range(ncores):
        ic = np.ascontiguousarray(
            ic_all[:, c * nimg:(c + 1) * nimg, :].reshape(75, nimg * 1024))
        in_maps.append({"ic": ic, "w1": w1h, "w2": w2h, "w3": w3h,
                        "w4": w4h, "b1": b1h, "b2": b2h, "b3": b3h,
                        "b4": b4h, "zz": zz})
    return in_maps


def _run(inputs, trace=False, nimg=512, ncores=NCORES):
    from concourse.bass_utils import run_bass_kernel_spmd

    key = (nimg, ncores)
    if key not in _CACHE:
        _CACHE[key] = _build_program(nimg)
    nc = _CACHE[key]

    in_maps = _prep_inputs(
        inputs["x"], inputs["W1"], inputs["b1"], inputs["W2"], inputs["b2"],
        inputs["W3"], inputs["b3"], inputs["W4"], inputs["b4"], nimg)

    res = run_bass_kernel_spmd(nc, in_maps, core_ids=list(range(ncores)),
                               trace=trace)
    feats = np.concatenate(
        [r["f"].transpose(2, 1, 0, 3).reshape(nimg, 8192)
         for r in res.results], axis=0)                          # [N, 8192]
    return feats, res


def kernel(**inputs):
    x = np.asarray(inputs["x"])
    n = x.shape[0]
    nimg = n // NCORES
    feats, _ = _run(inputs, trace=False, nimg=nimg)

    if int(np.asarray(inputs.get("is_local", 1))) == 0:
        return feats.astype(np.float32)

    batch_size = int(np.asarray(inputs["batch_size"]))
    seg = np.asarray(inputs["f_obj_to_img"]).astype(np.int64)
    nh = n // 2
    fake, real = feats[:nh], feats[nh:]
    counts = np.bincount(seg, minlength=batch_size).astype(np.float32)
    denom = np.maximum(counts, 1.0)[:, None]
    fsum = np.zeros((batch_size, 8192), np.float32)
    rsum = np.zeros((batch_size, 8192), np.float32)
    np.add.at(fsum, seg, fake)
    np.add.at(rsum, seg, real)
    favg = np.where((counts > 0)[:, None], fsum / denom, 0.0)
    ravg = np.where((counts > 0)[:, None], rsum / denom, 0.0)
    return np.concatenate([favg, ravg], axis=0).astype(np.float32)


# revision 3
# speedup vs baseline: 1.9774x; 1.3223x over previous
"""Trainium2 Bass kernel for nn_ContextualCritic (4-layer strided conv + segment mean).

Self-contained: kernel(**inputs) -> np.ndarray [2B, 8192].

Design (per core, data-parallel over 8 cores, 512 images each), bf16 matmuls:
 - L1 (3->64, 5x5 s2): host im2col to K=75, zero-padded to K=128; weights
   duplicated along M so the output lands twice in PSUM partitions (0-63 and
   64-127). One N=1024 activation per image (2-bank PSUM tile) amortizes the
   ACT pipeline-fill overhead.
 - L2 (64->128): 25 taps as interleaved K=64 matmul pairs on PE row groups
   (0,0)/(64,0) into two PSUM banks, plus the odd tap as one K=128 matmul
   with zeroed high weight rows; DVE adds banks, two ACTs (one per output
   column parity) write the L2 output in a column-phase, image-inner layout.
 - L3 (128->256), L4 (256->512): inputs stored column-phase + image-innermost
   so every conv tap's moving operand is a contiguous 16B-aligned stream
   (full-rate PE streaming); K=128 accumulation matmuls over taps; w4 stays
   resident in SBUF (loaded once).
 - All matmul operands bf16 (fp32 PSUM accumulate; FWL active), biases fp32,
   final features fp32.
 - Segment mean on host from [N,8192] features (sorted segment ids).
"""
import os
import numpy as np

BLK = 8        # images per Phase-A block
GRP = 32       # images per L4 group (N = GRP*16 = 512)
NCORES = 8

_CACHE = {}


def _build_program(nimg, debug=False):
    from concourse import bacc, mybir
    import concourse.tile as tile

    BF16 = mybir.dt.bfloat16
    F32 = mybir.dt.float32
    LRELU = mybir.ActivationFunctionType.Prelu

    nblk = nimg // BLK
    ngrp = nimg // GRP

    nc = bacc.Bacc(None, target_bir_lowering=False)

    icd = nc.dram_tensor("ic", [75, nimg * 1024], BF16, kind="ExternalInput")
    w1d = nc.dram_tensor("w1", [128, 128], BF16, kind="ExternalInput")
    zd = nc.dram_tensor("zz", [128, 10368], BF16, kind="ExternalInput")
    w2d = nc.dram_tensor("w2", [128, 25 * 128], BF16, kind="ExternalInput")
    w3d = nc.dram_tensor("w3", [128, 2 * 25 * 128], BF16, kind="ExternalInput")
    w4d = nc.dram_tensor("w4", [128, 50 * 512], BF16, kind="ExternalInput")
    b1d = nc.dram_tensor("b1", [128, 1], F32, kind="ExternalInput")
    b2d = nc.dram_tensor("b2", [128, 1], F32, kind="ExternalInput")
    b3d = nc.dram_tensor("b3", [128, 2], F32, kind="ExternalInput")
    b4d = nc.dram_tensor("b4", [128, 4], F32, kind="ExternalInput")
    # f[p, q, grp, (r c i)] -> channel co = q*128+p, feature co*16+r*4+c,
    # image grp*GRP+i
    fd = nc.dram_tensor("f", [128, 4, ngrp, 512], F32, kind="ExternalOutput")

    with tile.TileContext(nc) as tc:
        with tc.tile_pool(name="const", bufs=1) as cst, \
             tc.tile_pool(name="dram", bufs=1, space="DRAM") as drp:
            # per 8-image block gb: [16 r, 2 cpar, 8 c2, 8 img]
            l2od = drp.tile([128, nblk, 2048], BF16)

            w1t = cst.tile([128, 128], BF16)
            nc.sync.dma_start(w1t[:], w1d[:, :])
            w2t = cst.tile([128, 25 * 128], BF16)
            nc.sync.dma_start(w2t[:], w2d[:, :])
            w3t = cst.tile([128, 2 * 25 * 128], BF16)
            nc.sync.dma_start(w3t[:], w3d[:, :])
            w4t = cst.tile([128, 50 * 512], BF16)
            nc.sync.dma_start(w4t[:], w4d[:, :])
            b1t = cst.tile([128, 1], F32)
            nc.sync.dma_start(b1t[:], b1d[:, :])
            b2t = cst.tile([128, 1], F32)
            nc.sync.dma_start(b2t[:], b2d[:, :])
            b3t = cst.tile([128, 2], F32)
            nc.sync.dma_start(b3t[:], b3d[:, :])
            b4t = cst.tile([128, 4], F32)
            nc.sync.dma_start(b4t[:], b4d[:, :])
            a2t = cst.tile([128, 1], F32)
            nc.vector.memset(a2t[:], 0.2)

            # ---------------- Phase A: L1 + L2 ----------------
            with tc.tile_pool(name="pa", bufs=1) as pa, \
                 tc.tile_pool(name="paps", bufs=2, space="PSUM") as paps, \
                 tc.tile_pool(name="past", bufs=3) as past:
                icT = [pa.tile([128, BLK * 1024], BF16, name=f"ic{i}")
                       for i in range(2)]
                l2iT = [pa.tile([128, BLK, 36, 36], BF16, name=f"l2i{i}")
                        for i in range(2)]
                for i in range(2):
                    nc.sync.dma_start(icT[i][75:128, :], zd[0:53, 0:BLK * 1024])
                    nc.sync.dma_start(
                        l2iT[i][:].rearrange("p i r c -> p (i r c)"),
                        zd[:, 0:10368])

                def l1_img(ic, l2i, img):
                    ps = paps.tile([128, 2, 16, 32], F32, tag="l1ps")
                    for h in range(2):
                        nc.tensor.matmul(
                            ps[:, h, :, :], w1t[:, :],
                            ic[:, (2 * img + h) * 512:(2 * img + h + 1) * 512],
                            start=True, stop=True)
                    nc.scalar.activation(
                        l2i[:, img, 2:34, 2:34],
                        ps[:].rearrange("p h r c -> p (h r) c"),
                        LRELU, bias=b1t[:, :], alpha=a2t[:, :])

                def l2_psb(l2i, ob, psb):
                    j0 = 2 * psb
                    psA = paps.tile([128, 512], F32, tag="l2psA")
                    psB = paps.tile([128, 512], F32, tag="l2psB")
                    for i in range(12):
                        tA, tB = 2 * i, 2 * i + 1
                        ka, wa = tA // 5, tA % 5
                        kb, wb = tB // 5, tB % 5
                        nc.tensor.matmul(
                            psA[:, :], w2t[0:64, tA * 128:(tA + 1) * 128],
                            l2i[0:64, j0:j0 + 2, ka:ka + 32:2, wa:wa + 32:2],
                            start=(i == 0), stop=False)
                        nc.tensor.matmul(
                            psB[:, :], w2t[64:128, tB * 128:(tB + 1) * 128],
                            l2i[64:128, j0:j0 + 2, kb:kb + 32:2, wb:wb + 32:2],
                            start=(i == 0), stop=(i == 11),
                            tile_position=(64, 0))
                    # tap 24 as K=128 (high weight rows are zero on host)
                    nc.tensor.matmul(
                        psA[:, :], w2t[:, 24 * 128:25 * 128],
                        l2i[:, j0:j0 + 2, 4:36:2, 4:36:2],
                        start=False, stop=True)
                    tb = past.tile([128, 512], F32, tag="l2tb")
                    nc.vector.tensor_copy(tb[:], psB[:, :])
                    st = past.tile([128, 512], F32, tag="l2st")
                    nc.vector.tensor_tensor(st[:], psA[:, :], tb[:],
                                            op=mybir.AluOpType.add)
                    # st flat = (i, r, c2, two); write ob[r, two, c2, j0+i]
                    sv = st[:].rearrange("p (i r c two) -> p r c two i",
                                         i=2, r=16, c=8)
                    for two in range(2):
                        nc.scalar.activation(
                            ob[:, :, two, :, j0:j0 + 2], sv[:, :, :, two, :],
                            LRELU, bias=b2t[:, :], alpha=a2t[:, :])

                for blk in range(nblk):
                    ic = icT[blk % 2]
                    l2i = l2iT[blk % 2]
                    c0 = blk * BLK * 1024
                    nc.sync.dma_start(ic[0:64, :], icd[0:64, c0:c0 + BLK * 1024])
                    nc.sync.dma_start(ic[64:75, :], icd[64:75, c0:c0 + BLK * 1024])
                    ob = past.tile([128, 16, 2, 8, 8], BF16, tag="l2ob")
                    # interleave L1 image pairs with L2 psum blocks
                    l1_img(ic, l2i, 0)
                    l1_img(ic, l2i, 1)
                    l1_img(ic, l2i, 2)
                    l1_img(ic, l2i, 3)
                    l2_psb(l2i, ob, 0)
                    l1_img(ic, l2i, 4)
                    l1_img(ic, l2i, 5)
                    l2_psb(l2i, ob, 1)
                    l1_img(ic, l2i, 6)
                    l1_img(ic, l2i, 7)
                    l2_psb(l2i, ob, 2)
                    l2_psb(l2i, ob, 3)
                    nc.sync.dma_start(
                        l2od[:, blk, :],
                        ob[:].rearrange("p r t c i -> p (r t c i)"))

            # ---------------- Phase B: L3 + L4 ----------------
            # l3i: [20 r, 2 cpar, 10 c2, 8 img]; taps 16B-aligned, img-inner
            # l4i: [12 r, 2 cpar, 6 c2, 32 img]
            with tc.tile_pool(name="pb", bufs=1) as pb, \
                 tc.tile_pool(name="pbps", bufs=1, space="PSUM") as pbps, \
                 tc.tile_pool(name="pbst", bufs=3) as pbst:
                l3iT = [pb.tile([128, 20, 2, 10, BLK], BF16, name=f"l3i{i}")
                        for i in range(2)]
                l4iT = [pb.tile([128, 12, 2, 6, GRP], BF16, name=f"l4i{i}")
                        for i in range(2)]
                for i in range(2):
                    nc.sync.dma_start(
                        l3iT[i][:].rearrange("p r t c i -> p (r t c i)"),
                        zd[:, 0:20 * 2 * 10 * BLK])
                    nc.sync.dma_start(
                        l4iT[i][:].rearrange("p r t c i -> p (r t c i)"),
                        zd[:, 0:12 * 2 * 6 * GRP])
                l4ps = [pbps.tile([128, 4, 4, GRP], F32, name=f"l4ps{q}")
                        for q in range(4)]

                for grp in range(ngrp):
                    for sb4 in range(4):
                        gb = grp * 4 + sb4
                        l3i = l3iT[gb % 2]
                        lv = l2od[:, gb, :].rearrange(
                            "p (r t ci) -> p r t ci", r=16, t=2)
                        for two in range(2):
                            nc.sync.dma_start(
                                l3i[:, 2:18, two, 1:9, :].rearrange(
                                    "p r c i -> p r (c i)"),
                                lv[:, :, two, :])
                        for cp in range(2):
                            ps3 = pbps.tile([128, 8, 8, BLK], F32,
                                            tag=f"l3ps{cp}")
                            for tap in range(25):
                                kh, kw = tap // 5, tap % 5
                                nc.tensor.matmul(
                                    ps3[:, :, :, :],
                                    w3t[:, (cp * 25 + tap) * 128:
                                        (cp * 25 + tap + 1) * 128],
                                    l3i[:, kh:kh + 16:2, kw % 2,
                                        kw // 2:kw // 2 + 8, :],
                                    start=(tap == 0), stop=(tap == 24))
                            for two in range(2):
                                nc.scalar.activation(
                                    l4iT[cp][:, 2:10, two, 1:5,
                                             sb4 * BLK:(sb4 + 1) * BLK],
                                    ps3[:, :, two::2, :], LRELU,
                                    bias=b3t[:, cp:cp + 1], alpha=a2t[:, :])
                    # L4 over the 32-image group (w4 resident in SBUF)
                    for i4 in range(50):
                        cip, tap = i4 // 25, i4 % 25
                        kh, kw = tap // 5, tap % 5
                        for q in range(4):
                            nc.tensor.matmul(
                                l4ps[q][:, :, :, :],
                                w4t[:, i4 * 512 + q * 128:
                                    i4 * 512 + (q + 1) * 128],
                                l4iT[cip][:, kh:kh + 8:2, kw % 2,
                                          kw // 2:kw // 2 + 4, :],
                                start=(i4 == 0), stop=(i4 == 49))
                    for q in range(4):
                        fo = pbst.tile([128, 512], F32, tag="fo")
                        nc.scalar.activation(
                            fo[:], l4ps[q][:].rearrange("p r c i -> p (r c i)"),
                            LRELU, bias=b4t[:, q:q + 1], alpha=a2t[:, :])
                        nc.sync.dma_start(fd[:, q, grp, :], fo[:])
    nc.compile()
    return nc


def _prep_inputs(x, W1, b1, W2, b2, W3, b3, W4, b4, nimg):
    """Host preprocessing -> per-core in_maps (shared weight arrays)."""
    import ml_dtypes
    f32 = np.float32
    bf16 = ml_dtypes.bfloat16
    n = x.shape[0]
    ncores = n // nimg
    xpad = np.pad(np.asarray(x, dtype=f32), ((0, 0), (0, 0), (2, 2), (2, 2)))
    s = xpad.strides
    v = np.lib.stride_tricks.as_strided(
        xpad, shape=(n, 3, 5, 5, 32, 32),
        strides=(s[0], s[1], s[2], s[3], 2 * s[2], 2 * s[3]))
    # [75, n, 1024]
    ic_all = np.ascontiguousarray(
        v.transpose(1, 2, 3, 0, 4, 5).reshape(75, n, 1024)).astype(bf16)

    w1l = np.ascontiguousarray(
        np.asarray(W1, f32).transpose(1, 2, 3, 0).reshape(75, 64))
    w1h = np.zeros((128, 128), f32)
    w1h[0:75, 0:64] = w1l
    w1h[0:75, 64:128] = w1l
    zz = np.zeros((128, 10368), bf16)
    b1h = np.concatenate([b1, b1]).astype(f32).reshape(128, 1)

    w2h = np.zeros((128, 25 * 128), f32)
    for t in range(25):
        kh, kw = t // 5, t % 5
        lhs = np.asarray(W2, f32)[:, :, kh, kw].T                # [64,128]
        w2h[0:64, t * 128:(t + 1) * 128] = lhs
        if t < 24:
            w2h[64:128, t * 128:(t + 1) * 128] = lhs
    b2h = np.asarray(b2, f32).reshape(128, 1)

    w3h = np.zeros((128, 2 * 25 * 128), f32)
    for cp in range(2):
        for t in range(25):
            kh, kw = t // 5, t % 5
            w3h[:, (cp * 25 + t) * 128:(cp * 25 + t + 1) * 128] = \
                np.asarray(W3, f32)[cp * 128:(cp + 1) * 128, :, kh, kw].T
    b3h = np.ascontiguousarray(
        np.asarray(b3, f32).reshape(2, 128).T)                   # [128,2]

    w4h = np.zeros((128, 50 * 512), f32)
    for cip in range(2):
        for t in range(25):
            kh, kw = t // 5, t % 5
            i4 = cip * 25 + t
            w4h[:, i4 * 512:(i4 + 1) * 512] = \
                np.asarray(W4, f32)[:, cip * 128:(cip + 1) * 128, kh, kw].T
    b4h = np.ascontiguousarray(
        np.asarray(b4, f32).reshape(4, 128).T)                   # [128,4]

    w1h = w1h.astype(bf16)
    w2h = w2h.astype(bf16)
    w3h = w3h.astype(bf16)
    w4h = w4h.astype(bf16)

    in_maps = []
    for c in range(ncores):
        ic = np.ascontiguousarray(
            ic_all[:, c * nimg:(c + 1) * nimg, :].reshape(75, nimg * 1024))
        in_maps.append({"ic": ic, "w1": w1h, "w2": w2h, "w3": w3h,
                        "w4": w4h, "b1": b1h, "b2": b2h, "b3": b3h,
                        "b4": b4h, "zz": zz})
    return in_maps


def _run(inputs, trace=False, nimg=512, ncores=NCORES):
    from concourse.bass_utils import run_bass_kernel_spmd

    key = (nimg, ncores)
    if key not in _CACHE:
        _CACHE[key] = _build_program(nimg)
    nc = _CACHE[key]

    in_maps = _prep_inputs(
        inputs["x"], inputs["W1"], inputs["b1"], inputs["W2"], inputs["b2"],
        inputs["W3"], inputs["b3"], inputs["W4"], inputs["b4"], nimg)

    res = run_bass_kernel_spmd(nc, in_maps, core_ids=list(range(ncores)),
                               trace=trace)
    ngrp = nimg // GRP
    feats = np.concatenate(
        [r["f"].reshape(128, 4, ngrp, 4, 4, GRP)
         .transpose(2, 5, 1, 0, 3, 4).reshape(nimg, 8192)
         for r in res.results], axis=0)                          # [N, 8192]
    return feats, res


def kernel(**inputs):
    x = np.asarray(inputs["x"])
    n = x.shape[0]
    nimg = n // NCORES
    feats, _ = _run(inputs, trace=False, nimg=nimg)

    if int(np.asarray(inputs.get("is_local", 1))) == 0:
        return feats.astype(np.float32)

    batch_size = int(np.asarray(inputs["batch_size"]))
    seg = np.asarray(inputs["f_obj_to_img"]).astype(np.int64)
    nh = n // 2
    fake, real = feats[:nh], feats[nh:]
    counts = np.bincount(seg, minlength=batch_size).astype(np.float32)
    denom = np.maximum(counts, 1.0)[:, None]
    fsum = np.zeros((batch_size, 8192), np.float32)
    rsum = np.zeros((batch_size, 8192), np.float32)
    np.add.at(fsum, seg, fake)
    np.add.at(rsum, seg, real)
    favg = np.where((counts > 0)[:, None], fsum / denom, 0.0)
    ravg = np.where((counts > 0)[:, None], rsum / denom, 0.0)
    return np.concatenate([favg, ravg], axis=0).astype(np.float32)
